# revision 35
# baseline (speedup 1.0000x reference)
"""Chamfer loss kernel for Trainium2 (8 NeuronCores, SPMD).

Problem: B=4, N=M=8192, D=64 (fp32 in / fp32 scalar out).
  dist[b,n,m] = ||f[b,n] - f_[b,m]||^2
  out = mean_b( mean_n min_m dist + mean_m min_n dist )

Sharding: core c handles batch c//2, row-half c%2 (4096 rows x 8192 cols
of the distance matrix). Each core computes complete row-mins for its
4096 rows and partial col-mins (over its rows) for all 8192 cols; host
combines partials (min over the 2 cores per batch + means).

Device dataflow per core (measured ~258.5us HW, vs 282.6us baseline):
  - matmul (fp16, K=66): lhsT = [-2*f^T ; p ; 1], rhs = [f_^T ; 1 ; q-SHIFT]
    so PSUM = dist - SHIFT directly. PSUM is managed as [128,1024] units
    (bufs=4) so the PE has up to 3 units of runway. PE is the critical
    resource: ~230us active at the pinned 1.2 GHz clock (the HAM never
    un-throttles on this part; a warmup burst was tried and does nothing),
    with LDWEIGHTS un-hidable (K=66 occupies row strips 0-2 for every
    tile, so the pull-ahead window never applies).
  - A hand-authored custom DVE op (CHAMFER_MINMIN) does BOTH min passes in
    one read: out[:, :W] = min(in0, C) is the running col-min, and a
    row-min accumulator rides the datapath (block-3 CURR_ALU_OUT feedback
    reading raw Src0, inexpressible in the stock Spec DSL) and is emitted
    by two extra drain FSM states as 2 trailing output columns. The 2x_1p
    program is hand-written (stock-TT-style packed halves); byte36[7:6]
    perf_max=1 enables it. The stock 0xf7 accum-read companion reads
    garbage in 2x mode, which is why the row-min travels via the output
    stream instead of accum_out.
  - C accumulators are [128, 2*(1024+2)]: two half-group segments, each
    with 2 trailing pad columns, ping-ponged per n-tile. ACT-path tiles:
    ScalarE copies each PSUM unit to fp16 feed halves (~1.1us each), DVE
    runs the fused op per half at 2x (~0.6us). K_PSUM=32 of the 128 tiles
    skip ScalarE entirely: the DVE fused op reads PSUM fp32 at 1x
    (~1.2us/half). This balances ACT ~213us / DVE ~212us under PE ~230us.
  - Row-min pads are staged into a [128, 256] SBUF strip by tiny strided
    GpSimd copies (in-FIFO ordering = no WAR exposure; DMA-per-tile
    eviction serializes the whole pipeline and must not come back) and
    shipped once at the end.
  - Final col stores fan 2MB across the sync/gpsimd/scalar DMA queues
    (all share one AXI port, so the tail is ~14us bandwidth-bound).
Known dead ends (measured): g-major loop order (C RAW chain serializes
the DVE: 474-689us); eviction via per-tile DMA on sync (298-342us) or on
the scalar ring (412us, head-of-line blocks ACT); InstMatmult.ldweights
flag (False crashes the device without a paired InstLdweights, ignored
with one); PE HAM warmup bursts (clock is pinned); coarse [128,2048]
PSUM tiles (PE stalls, 298us). Device throttles ~20% under sustained
back-to-back runs; compare timings only after a ~2min cooldown.
"""

import os

import numpy as np

import concourse.bass as bass
import concourse.dve_ops as DO
import concourse.mybir as mybir
import concourse.tile as tile
from concourse import bacc
from concourse.bass import ts
from concourse.bass_utils import run_bass_kernel_spmd
from concourse.dve_spec import C0, AluOp as SpecAluOp, Spec, Src0, Src1, minn
from concourse.dve_uop import (
    AluInp,
    AluOp,
    DelayInp,
    DveOpSpec,
    InpSel,
    OutPath,
    OutSel,
    Trigger,
    UopConfig,
)

# --------------------------------------------------------------------------
# Custom DVE op: fused col-min tensor_tensor + row-min reduction
# --------------------------------------------------------------------------

OP_NAME = "CHAMFER_MINMIN"
DRAIN_A = 8  # drain cycles so the held row-min ripples blk3->blk7 (>=4)


def _reference(in0, in1, s0, s1, imm2):
    in0 = np.asarray(in0, np.float32)
    body = np.minimum(in0, np.asarray(in1, np.float32))
    rm = in0.reshape(in0.shape[0], -1).min(axis=-1, keepdims=True)
    if isinstance(s0, np.ndarray):
        rm = np.minimum(np.asarray(s0, np.float32).reshape(-1, 1), rm)
    else:
        rm = np.minimum(float(s0), rm)
    return np.concatenate([body, rm, rm], axis=1)


_SPEC = Spec(
    body=minn(Src0, Src1),
    accum=SpecAluOp.MIN,
    accum_init=C0,
    reference=_reference,
)


def _build_1x():
    def common(u: UopConfig):
        u.enable_input(InpSel.SRC_0, 1)
        u.enable_input(InpSel.SRC_1, 2)
        u.enable_input(InpSel.CONST_0, 3)
        b = u.datapath_config
        b[0].enable_alu(AluOp.MIN, AluInp.PREV_DELAY_0, AluInp.PREV_DELAY_1)
        b[0].pass_through_delay(0, 1, 2)
        b[1].enable_delay_from_src(DelayInp.PREV_ALU_OUT, 0)
        b[1].pass_through_delay(1, 2)
        for i in range(2, 8):
            b[i].pass_through_alu()
            b[i].pass_through_delay(0, 1, 2)
        return u

    seed = common(UopConfig())
    seed.datapath_config[1].enable_alu(
        AluOp.BYPASS, AluInp.PREV_DELAY_2, AluInp.PREV_DELAY_2
    )
    seed.repeat_count = 1
    seed.trigger = (Trigger.COUNT, Trigger.NONE, Trigger.NONE)
    seed.next_uop = (1, 0, 0)

    steady = common(UopConfig())
    steady.datapath_config[1].enable_alu(
        AluOp.MIN, AluInp.CURR_ALU_OUT, AluInp.PREV_DELAY_0
    )
    steady.trigger = (Trigger.SRC_TENSOR_DONE, Trigger.NONE, Trigger.NONE)
    steady.next_uop = (2, 0, 0)
    steady.require_inp0 = 1
    steady.require_inp1 = 1
    steady.enable_output(OutSel.DELAY_0, OutPath.WR0_LO)

    def drain(u: UopConfig):
        b = u.datapath_config
        b[1].enable_alu(AluOp.BYPASS, AluInp.CURR_ALU_OUT, AluInp.CURR_ALU_OUT)
        for i in range(2, 8):
            b[i].pass_through_alu()
        u.trigger = (Trigger.COUNT, Trigger.NONE, Trigger.NONE)
        return u

    drain_a = drain(UopConfig())
    drain_a.repeat_count = DRAIN_A
    drain_a.next_uop = (3, 0, 0)

    drain_b = drain(UopConfig())
    drain_b.repeat_count = 2
    drain_b.next_uop = (0, 0, 0)
    drain_b.enable_output(OutSel.ALU_OUT, OutPath.WR0_LO)

    return [seed, steady, drain_a, drain_b]


def _build_2x():
    def common(u: UopConfig):
        u.enable_input(InpSel.SRC_0, 0)
        u.enable_input(InpSel.SRC_1, 1)
        u.enable_input(InpSel.SRC_0_HI, 2)
        u.enable_input(InpSel.SRC_1_HI, 3)
        u.enable_input(InpSel.CONST_0, 4)
        b = u.datapath_config
        b[0].enable_alu(AluOp.MIN, AluInp.PREV_ALU_OUT, AluInp.PREV_DELAY_0)
        b[0].pass_through_delay(1, 2, 3)
        b[0].enable_delay_from_src(DelayInp.PREV_ALU_OUT, 4)
        b[1].enable_alu(AluOp.MIN, AluInp.PREV_DELAY_1, AluInp.PREV_DELAY_2)
        b[1].enable_delay_from_src(DelayInp.PREV_ALU_OUT, 0)
        b[1].pass_through_delay(1, 3, 4)
        b[2].enable_alu(AluOp.MIN, AluInp.PREV_DELAY_4, AluInp.PREV_DELAY_1)
        b[2].enable_delay_from_src(DelayInp.PREV_ALU_OUT, 2)
        b[2].pass_through_delay(0, 3)
        b[3].pass_through_delay(0, 2, 3)
        for i in range(4, 8):
            b[i].pass_through_alu()
            b[i].pass_through_delay(0, 2, 3)
        return u

    seed = common(UopConfig())
    seed.datapath_config[3].enable_alu(
        AluOp.BYPASS, AluInp.PREV_DELAY_3, AluInp.PREV_DELAY_3
    )
    seed.repeat_count = 1
    seed.trigger = (Trigger.COUNT, Trigger.NONE, Trigger.NONE)
    seed.next_uop = (1, 0, 0)

    steady = common(UopConfig())
    steady.datapath_config[3].enable_alu(
        AluOp.MIN, AluInp.CURR_ALU_OUT, AluInp.PREV_ALU_OUT
    )
    steady.trigger = (Trigger.SRC_TENSOR_DONE, Trigger.NONE, Trigger.NONE)
    steady.next_uop = (2, 0, 0)
    steady.require_inp0 = 1
    steady.require_inp1 = 1
    steady.enable_output(OutSel.DELAY_0, OutPath.WR0_LO)
    steady.enable_output(OutSel.DELAY_2, OutPath.WR0_HI)

    def drain(u: UopConfig):
        b = u.datapath_config
        b[3].enable_alu(AluOp.BYPASS, AluInp.CURR_ALU_OUT, AluInp.CURR_ALU_OUT)
        for i in range(4, 8):
            b[i].pass_through_alu()
        u.trigger = (Trigger.COUNT, Trigger.NONE, Trigger.NONE)
        return u

    drain_a = drain(UopConfig())
    drain_a.repeat_count = DRAIN_A
    drain_a.next_uop = (3, 0, 0)

    drain_b = drain(UopConfig())
    drain_b.repeat_count = 1
    drain_b.next_uop = (0, 0, 0)
    drain_b.enable_output(OutSel.ALU_OUT, OutPath.WR0_LO)
    drain_b.enable_output(OutSel.ALU_OUT, OutPath.WR0_HI)

    return [seed, steady, drain_a, drain_b]


class _FusedOp:
    name = OP_NAME
    spec = _SPEC
    subdim = False

    def __init__(self):
        self._cache = {}

    def compile(self, ver):
        if ver in self._cache:
            return self._cache[ver]
        assert ver == "v3", f"only TRN2/v3 supported, got {ver}"
        s = DveOpSpec(
            name=self.name,
            opcode=DO.get_dve_sub_opcode(self.name),
            uops=_build_1x(),
            uops_2x=_build_2x(),
            rd1_en=True,
            perf_max=1,
        )
        s.validate(ver)
        self._cache[ver] = s
        return s


def _register():
    if OP_NAME in DO._SUB_OPCODE_FOR_NAME:
        return next(op for op in DO.OPS if op.name == OP_NAME)
    op = _FusedOp()
    DO.OPS.append(op)
    DO._SUB_OPCODE_FOR_NAME[OP_NAME] = DO._CUSTOM_DVE_ROW_BASE + len(DO.OPS) - 1
    DO.CUSTOM_DVE_SPECS[OP_NAME] = _SPEC
    return op


FUSED_OP = _register()


def emit_fused(nc, out, in0, in1, s0):
    inst = nc.vector._custom_dve(FUSED_OP, out=out, in0=in0, in1=in1, s0=s0, s1=0.0)
    inst.ins.perf_max = 1  # BassInstruction wraps the rust instr as .ins
    return inst


# --------------------------------------------------------------------------
# Chamfer kernel
# --------------------------------------------------------------------------

B, N, M, D = 4, 8192, 8192, 64
N_CORES = 8
ROWS = N // 2          # rows per core (half a batch)
SHIFT = 48.0
BIGVAL = 60000.0       # row-min accumulator seed (fp16-safe "+inf")

P = 128                # n-tile height (PSUM partitions)
MB = 512               # m-block width (one PSUM bank of fp32)
GROUP = 4              # m-blocks per PSUM group tile ([128, 2048] = 4 banks)
PAD = 2                # row-min pad columns appended to each C group

# every k-th eligible tile (i>0) goes PSUM-direct on the DVE (no ACT copy)
K_PSUM = int(os.environ.get("CHAMFER_K_PSUM", "32"))

LAST_RESULTS = None    # test.py reads exec_time_ns / profile from here


def _build_program(rows=ROWS, cols=M):
    n_tiles = rows // P
    m_groups = cols // (MB * GROUP)
    GW = MB * GROUP        # feed-group width (2048)
    K = D + 2

    f16 = mybir.dt.float16
    f32 = mybir.dt.float32

    # choose PSUM-direct tiles: spread K_PSUM of the i>0 tiles evenly
    # (i-major linear index: lin = i * m_groups + g; skip i == 0)
    n_lin = n_tiles * m_groups
    eligible = [t for t in range(m_groups, n_lin)]
    psum_path = set()
    if K_PSUM > 0:
        stride = len(eligible) / K_PSUM
        psum_path = {eligible[min(len(eligible) - 1, int(j * stride))]
                     for j in range(K_PSUM)}

    nc = bacc.Bacc()
    lhs_d = nc.dram_tensor("lhs", [K, rows], f16, kind="ExternalInput")
    rhs_d = nc.dram_tensor("rhs", [K, cols], f16, kind="ExternalInput")
    row_d = nc.dram_tensor("rowacc", [P, n_tiles * m_groups * 2], f16,
                           kind="ExternalOutput")
    col_d = nc.dram_tensor("colmins", [P, cols], f16, kind="ExternalOutput")

    with tile.TileContext(nc) as tc:
        with (
            tc.tile_pool(name="const", bufs=1) as const_pool,
            tc.tile_pool(name="feed", bufs=8) as feed_pool,
            tc.tile_pool(name="psum", bufs=4, space="PSUM") as psum_pool,
        ):
            lhs_sb = const_pool.tile([K, rows], f16)
            rhs_sb = const_pool.tile([K, cols], f16)
            # chunked loads, ordered so the first matmul (lhs cols 0:128 +
            # rhs cols 0:512) gates on the first two DMAs, not the whole train
            nc.sync.dma_start(lhs_sb[:, 0:P], lhs_d[:, 0:P])
            for c in range(0, GW, MB):
                nc.sync.dma_start(rhs_sb[:, c:c + MB], rhs_d[:, c:c + MB])
            # the whole first i-iteration needs all 4 rhs groups; lhs beyond
            # tile 0 isn't needed until ~18us in, so ship rhs groups 1-3
            # before the bulk lhs chunks (removes a ~3.6us PE startup stall)
            for c in range(GW, cols, GW):
                nc.sync.dma_start(rhs_sb[:, c:c + GW], rhs_d[:, c:c + GW])
            nc.sync.dma_start(lhs_sb[:, P:GW], lhs_d[:, P:GW])
            for c in range(GW, rows, GW):
                e = min(c + GW, rows)
                nc.sync.dma_start(lhs_sb[:, c:e], lhs_d[:, c:e])

            # col-min accumulators, ping-pong pair per m-group so the pad
            # eviction DMA of tile i never WAR-blocks tile i+1's op.
            # Split-pad layout: two half-group segments of [HW cols + PAD
            # row-min cols] each, so PSUM-direct half-ops have their own pads.
            HW = GW // 2
            SEG = HW + PAD
            NROT = 2
            Cs = [
                [
                    const_pool.tile([P, 2 * SEG], f16, name=f"C{g}_{s}")
                    for s in range(NROT)
                ]
                for g in range(m_groups)
            ]

            rowstage = const_pool.tile([P, n_lin * 2], f16)

            for i in range(n_tiles):
                lhs_i = lhs_sb[:, ts(i, P)]
                for g in range(m_groups):
                    lin = i * m_groups + g
                    Cg_out = Cs[g][i % NROT]
                    Cg_in = Cs[g][(i + NROT - 1) % NROT]
                    # two half-group PSUM units -> finer PE/consumer pipeline
                    # (PE gets up to 3 units of runway instead of 1 tile)
                    pss = []
                    for h in range(2):
                        psh = psum_pool.tile([P, HW], f32, name="ps")
                        for jj in range(2):
                            nc.tensor.matmul(
                                psh[:, ts(jj, MB)], lhs_i,
                                rhs_sb[:, ts(g * GROUP + 2 * h + jj, MB)],
                                start=True, stop=True,
                            )
                        pss.append(psh)
                    if lin in psum_path:
                        # DVE reads PSUM directly (1x program): drain + both
                        # min passes in one op per half, ScalarE untouched
                        for h in range(2):
                            emit_fused(
                                nc,
                                out=Cg_out[:, h * SEG:(h + 1) * SEG],
                                in0=pss[h][:],
                                in1=Cg_in[:, h * SEG:h * SEG + HW],
                                s0=BIGVAL,
                            )
                    else:
                        sb = feed_pool.tile([P, GW], f16)
                        nc.scalar.copy(sb[:, 0:HW], pss[0][:])
                        nc.scalar.copy(sb[:, HW:GW], pss[1][:])
                        for h in range(2):
                            in0 = sb[:, h * HW:(h + 1) * HW]
                            emit_fused(
                                nc,
                                out=Cg_out[:, h * SEG:(h + 1) * SEG],
                                in0=in0,
                                in1=(in0 if i == 0
                                     else Cg_in[:, h * SEG:h * SEG + HW]),
                                s0=BIGVAL,
                            )
                    # stage this tile's row-min pads (first pad col of each
                    # half, stride SEG) with a tiny DVE copy: runs in-order
                    # on the DVE FIFO, so the next op on this C buffer can
                    # never overwrite the pads early; one bulk DMA ships the
                    # staging tensor at the end
                    nc.gpsimd.tensor_copy(
                        rowstage[:, 2 * lin:2 * lin + 2],
                        Cg_out[:, HW:HW + SEG + 1:SEG],
                    )

            nc.sync.dma_start(row_d[:], rowstage[:])
            # final col-min stores: fan the 2MB across the engine DMA queues,
            # one store per half-segment (skips the pad columns)
            lastrot = (n_tiles - 1) % NROT
            engines = [nc.sync, nc.gpsimd, nc.scalar, nc.sync]
            k = 0
            for g in range(m_groups):
                for h in range(2):
                    engines[k % len(engines)].dma_start(
                        col_d[:, g * GW + h * HW:g * GW + (h + 1) * HW],
                        Cs[g][lastrot][:, h * SEG:h * SEG + HW],
                    )
                    k += 1



    nc.finalize()
    return nc


_PROGRAM_CACHE = {}


def _get_program(rows=ROWS, cols=M):
    key = (rows, cols, K_PSUM)
    if key not in _PROGRAM_CACHE:
        _PROGRAM_CACHE[key] = _build_program(rows, cols)
    return _PROGRAM_CACHE[key]


def _prep_core_inputs(f, f_, core):
    """Host-side shard + layout: build augmented lhs/rhs for one core."""
    b, h = divmod(core, 2)
    fh = f[b, h * ROWS : (h + 1) * ROWS]          # [ROWS, D]
    g = f_[b]                                     # [M, D]
    p = np.einsum("nd,nd->n", fh, fh, dtype=np.float32)
    q = np.einsum("md,md->m", g, g, dtype=np.float32)

    K = D + 2
    lhs = np.empty((K, ROWS), np.float16)
    lhs[:D] = (-2.0 * fh.T).astype(np.float16)
    lhs[D] = p.astype(np.float16)
    lhs[D + 1] = 1.0

    rhs = np.empty((K, M), np.float16)
    rhs[:D] = g.T.astype(np.float16)
    rhs[D] = 1.0
    rhs[D + 1] = (q - SHIFT).astype(np.float16)
    return {"lhs": lhs, "rhs": rhs}


def kernel(f, f_):
    global LAST_RESULTS
    f = np.asarray(f, dtype=np.float32)
    f_ = np.asarray(f_, dtype=np.float32)

    in_maps = [_prep_core_inputs(f, f_, c) for c in range(N_CORES)]
    nc = _get_program()
    res = run_bass_kernel_spmd(
        nc,
        in_maps,
        list(range(N_CORES)),
        trace=bool(int(os.environ.get("CHAMFER_TRACE", "0"))),
    )
    LAST_RESULTS = res

    total = 0.0
    for b in range(B):
        r0 = res.results[2 * b]
        r1 = res.results[2 * b + 1]
        # rowacc[p, (i*4+g)*2+h] = row-min over group-half (g, h) for row
        # i*128 + p; fold the 8 group-half slots per n-tile
        def _rm(r):
            a = r["rowacc"].astype(np.float32)      # [128, n_tiles*8]
            nt = a.shape[1] // 8
            a = a.reshape(128, nt, 8).min(axis=2)   # [128, n_tiles]
            return a.T.reshape(-1)                  # row i*128+p
        rm = np.concatenate([_rm(r0), _rm(r1)]) + SHIFT
        cm = (
            np.minimum(
                r0["colmins"].astype(np.float32).min(axis=0),
                r1["colmins"].astype(np.float32).min(axis=0),
            )
            + SHIFT
        )
        total += rm.mean() + cm.mean()
    return np.asarray(total / B, dtype=np.float32)
